# revision 12
# baseline (speedup 1.0000x reference)
"""Trainium2 Bass kernel for nn_CustomCIFAR10Model.

Math (reference):
    xf = x.reshape(B, D)
    part2[b,d] = cos(xf[b,d]) * Sa[d] + sin(xf[b,d]) * Sb[d]
        where Sa[d] = sum_i a[i,d,0], Sb[d] = sum_i b[i,d,0]
    part1 = sum(w[1:]*n[1:] + w[:-1]*n[:-1])            (scalar)
    out = (part1 + part2) @ fc_w.T + fc_b               [B, NCLS]

The heavy part is reading a and b once to column-sum them: memory-bound.
Sharding: columns (d) split across 8 cores, 384 each. Every core
column-sums its a/b slice (PE matmuls), builds z = cos*Sa + sin*Sb for
its d-slice, and contracts against its fc_w columns, yielding partial
cos/sin [NCLS, B] outputs. Host sums the 16 partials and adds
part1/bias.

a/b/x are cast to bf16 on the host (tolerance is 2e-2; measured error
~4e-3): halves the dominant HBM traffic vs f32 and doubles the PE
matmul rate. The column-sum matmuls use an all-ones [128,128] bf16
stationary so the [128, DW] output lands on all 128 PSUM partitions
(identical rows): a [1, DW] output would serialize on a single PSUM
write port, capping the stream at ~380ns/chunk instead of ~160ns.
Sa is then moved onto partitions with a one-hot e0 matmul in bf16
(stationary = SBUF copy of the sum rows, moving = e0 picks row 0).

Each group DMA reads one fully contiguous DRAM block (its own dram
tensor, packed partition-major on the host): a strided source costs
~20% of per-engine DMA bandwidth. All input DMAs ride one HWDGE queue
(sync) FIFO: a0 (4 chunks, so the PE starts early) -> fwt -> xt ->
rest of a -> b (uniform 8-chunk b groups keep 6KB packets; small
trailing packets measured ~3x slower).

Trig prep is spread over three engines so no single queue serializes
the overlap window: ACT does the x*(1/2pi) scale copies + the 6 Sins,
DVE does the sin-side magic round/sub (+ all finish-side casts/muls),
GpSimd does the cos-side round/sub (cos is needed first, but GpSimd
at ~2.6 cyc/elem still lands it before the a-row sums are ready).

The PE HAM clock gate starts every kernel at 1.2 GHz and ramps to
2.4 GHz only after ~3.4us of sustained activity, and droops back
during idle gaps: dummy matmuls run during the preamble dead time and
as fillers in the early DMA gaps, targeting the row-sum banks before
their real accumulation groups start (start=True resets them).

cos and sin contract into separate PSUM banks: the cos store (copy +
DMA on the scalar queue) is fully hidden under the b stream, and only
the sin store remains on the tail.

HW Sin only accepts [-pi, pi]: range-reduce t = x/(2pi), r = t - round(t)
via the fp32 magic-number trick, then Sin(2pi*r); cos shifts t by +1/4.
"""

import numpy as np

B = 512
D = 3072
NCLS = 100
P = 128
NCORES = 8
DW = D // NCORES          # 384 columns per core
NSUB = DW // P            # 3 d-subtiles of 128
NCH = D // P              # 24 row-chunks of a/b slice
ROWSP = P
DPAD = D
NCHP = DPAD // ROWSP      # 24
GROUPS_A = [4, 8, 8, 4]
GROUPS_B = [8, 8, 6, 2]   # small tail: little matmul work after last byte
NWARM = 11                # preamble dummy matmuls to ramp the PE clock
FILLERS = [8, 3, 3]       # keep-warm matmuls after a0/a1/a2 matmul bursts
H = B // 2

_STATE = {}


def _build():
    """Build + bacc-compile the SPMD Bass program (once per process)."""
    import concourse.bacc as bacc
    import concourse.mybir as mybir
    import concourse.tile as tile

    f32 = mybir.dt.float32
    bf16 = mybir.dt.bfloat16
    nc = bacc.Bacc(
        "TRN2", target_bir_lowering=False, debug=False, num_devices=NCORES
    )

    grp_srcs = [[], []]
    for ti, sizes in enumerate((GROUPS_A, GROUPS_B)):
        for gi, n in enumerate(sizes):
            grp_srcs[ti].append(
                nc.dram_tensor(
                    f"{'ab'[ti]}{gi}", [ROWSP, n * DW], bf16, kind="ExternalInput"
                )
            )
    xt_s = nc.dram_tensor("xt_s", [P, NSUB * B], bf16, kind="ExternalInput")
    fwt_s = nc.dram_tensor("fwt_s", [P, NSUB * NCLS], bf16, kind="ExternalInput")
    out_c_cb = nc.dram_tensor("out_c", [P, B], bf16, kind="ExternalOutput")
    out_s_cb = nc.dram_tensor("out_s", [P, B], bf16, kind="ExternalOutput")

    INV2PI = float(1.0 / (2.0 * np.pi))
    TWO_PI = float(2.0 * np.pi)
    MAGIC = float(1.5 * 2.0**23)
    add_op = mybir.AluOpType.add
    sub_op = mybir.AluOpType.subtract
    Sin = mybir.ActivationFunctionType.Sin
    Copy = mybir.ActivationFunctionType.Copy

    with tile.TileContext(nc) as tc:
        with (
            tc.tile_pool(name="chunks", bufs=6) as chunk_pool,
            tc.tile_pool(name="consts", bufs=1) as const_pool,
            tc.tile_pool(name="xwork", bufs=1) as x_pool,
            tc.tile_pool(name="ps", bufs=2, space="PSUM") as psum_pool,
            tc.tile_pool(name="psrow", bufs=1, space="PSUM") as psum_row_pool,
            tc.tile_pool(name="psout", bufs=1, space="PSUM") as psum_out_pool,
        ):
            ones128 = const_pool.tile([P, P], bf16, name="ones128")
            nc.vector.memset(ones128[:], 1.0)
            e0 = const_pool.tile([P, 1], bf16, name="e0")
            nc.vector.memset(e0[:], 0.0)
            nc.vector.memset(e0[0:1, 0:1], 1.0)
            zero = const_pool.tile([P, 1], f32, name="zerob")
            nc.vector.memset(zero[:], 0.0)
            wsrc = const_pool.tile([P, DW], bf16, name="wsrc")
            nc.vector.memset(wsrc[:], 0.0)
            out_sb = {}
            for nm in ("c0", "c1", "s0", "s1"):
                t = const_pool.tile([P, H], bf16, name=f"out_sb_{nm}")
                nc.vector.memset(t[:], 0.0)
                out_sb[nm] = t
            # Dummy Sin so the Sin table set loads once at kernel start.
            warm = const_pool.tile([P, 1], f32, name="warm")
            nc.scalar.activation(warm[:], zero[:], Sin, bias=zero[:])

            rows = []
            for ti in range(2):
                psr = psum_row_pool.tile(
                    [P, DW], f32, name=f"psr{ti}", tag=f"psr{ti}"
                )
                rows.append(psr)
            emitted = [0, 0]

            # PE clock ramp: dummy matmuls into rows[0] while the PE would
            # otherwise idle; the first real matmul's start=True resets.
            for _ in range(NWARM):
                nc.tensor.matmul(
                    rows[0][:], ones128[:], wsrc[:], start=True, stop=True
                )

            def fillers(k):
                """Keep-warm matmuls into rows[1]; emitted only before the
                b stream's first real matmul (whose start=True resets)."""
                for _ in range(k):
                    nc.tensor.matmul(
                        rows[1][:], ones128[:], wsrc[:], start=True, stop=True
                    )

            def load_group(ti, gi, n):
                """One DMA for one contiguous group + its matmuls."""
                ch = chunk_pool.tile(
                    [ROWSP, n, DW], bf16, name=f"ch{ti}_{gi}", tag="chunk"
                )
                nc.sync.dma_start(out=ch[:], in_=grp_srcs[ti][gi][:])
                for j in range(n):
                    nc.tensor.matmul(
                        rows[ti][:],
                        ones128[0:ROWSP, :],
                        ch[:, j, :],
                        start=(emitted[ti] == 0),
                        stop=(emitted[ti] == NCHP - 1),
                    )
                    emitted[ti] += 1

            load_group(0, 0, GROUPS_A[0])
            fwt = x_pool.tile([P, NSUB, NCLS], bf16, name="fwt")
            nc.sync.dma_start(out=fwt[:], in_=fwt_s[:])
            xt = x_pool.tile([P, NSUB, B], bf16, name="xt")
            nc.sync.dma_start(out=xt[:], in_=xt_s[:])
            fillers(FILLERS[0])
            for gi, n in enumerate(GROUPS_A):
                if gi:
                    load_group(0, gi, n)
                    if gi < len(FILLERS):
                        fillers(FILLERS[gi])

            # Trig on x while a/b stream: r = t - round(t) (magic trick),
            # then Sin(2pi*r); cos shifts t by +1/4 before rounding.
            # Sin writes bf16 directly (matmul moving operand, no cast op).
            # Trig on x while a/b stream: r = t - round(t) (magic trick),
            # then Sin(2pi*r); cos shifts t by +1/4 before rounding. The
            # scale copies ride ACT, round/sub ride DVE (GpSimd tensor ops
            # measured ~7us each and starve concurrent DVE ops; mod is not
            # in the DVE ISA). All cos prep first: finish_tensor(0)
            # consumes cos early, sin is only needed at the tail.
            sins = []
            coss = []
            prep = []
            for sub in range(NSUB):
                tc_t = x_pool.tile([P, B], f32, name=f"tc{sub}", tag=f"tc{sub}")
                nc.scalar.activation(
                    tc_t[:], xt[:, sub, :], Copy, bias=0.25, scale=INV2PI
                )
                kc_t = x_pool.tile([P, B], f32, name=f"kc{sub}", tag=f"kc{sub}")
                nc.vector.tensor_scalar(kc_t[:], tc_t[:], MAGIC, MAGIC, add_op, sub_op)
                nc.vector.tensor_sub(tc_t[:], tc_t[:], kc_t[:])
                prep.append(tc_t)
            for sub in range(NSUB):
                ts_t = x_pool.tile([P, B], f32, name=f"ts{sub}", tag=f"ts{sub}")
                nc.scalar.activation(
                    ts_t[:], xt[:, sub, :], Copy, bias=0.0, scale=INV2PI
                )
                ks_t = x_pool.tile([P, B], f32, name=f"ks{sub}", tag=f"ks{sub}")
                nc.vector.tensor_scalar(ks_t[:], ts_t[:], MAGIC, MAGIC, add_op, sub_op)
                nc.vector.tensor_sub(ts_t[:], ts_t[:], ks_t[:])
                prep.append(ts_t)
            for sub in range(NSUB):
                cosv = x_pool.tile([P, B], bf16, name=f"cos{sub}", tag=f"cos{sub}")
                nc.scalar.activation(
                    cosv[:], prep[sub][:], Sin, bias=zero[:], scale=TWO_PI
                )
                coss.append(cosv)
            for sub in range(NSUB):
                sinv = x_pool.tile([P, B], bf16, name=f"sin{sub}", tag=f"sin{sub}")
                nc.scalar.activation(
                    sinv[:], prep[NSUB + sub][:], Sin, bias=zero[:], scale=TWO_PI
                )
                sins.append(sinv)

            out_cs = [
                psum_out_pool.tile([NCLS, B], f32, name=f"out_ps{ti}", tag=f"out{ti}")
                for ti in range(2)
            ]

            def finish_tensor(ti, vals):
                """Copy the (identical-row) sum block to SBUF as bf16 per
                128-subtile, pull row 0 onto partitions via a one-hot
                matmul, scale the SMALL fwt tiles by it (fwt[d,c]*S[d]),
                and contract (fwt*S).T @ trig into out_cs[ti]."""
                for sub in range(NSUB):
                    rsb = const_pool.tile(
                        [P, P], bf16, name=f"rsb{ti}_{sub}", tag=f"rsb{ti}{sub}"
                    )
                    nc.vector.tensor_copy(
                        rsb[:], rows[ti][:, sub * P : (sub + 1) * P]
                    )
                    ps = psum_pool.tile([P, 1], f32, name=f"ps{ti}_{sub}", tag="ps")
                    nc.tensor.matmul(
                        ps[:], rsb[:], e0[:], start=True, stop=True
                    )
                    fws = x_pool.tile(
                        [P, NCLS], bf16, name=f"fws{ti}_{sub}", tag=f"fws{ti}{sub}"
                    )
                    nc.vector.tensor_scalar_mul(fws[:], fwt[:, sub, :], ps[:])
                    nc.tensor.matmul(
                        out_cs[ti][:],
                        fws[:],
                        vals[sub][:],
                        start=(sub == 0),
                        stop=(sub == NSUB - 1),
                    )

            # a finishes mid-stream: its cos-side work + store overlap the
            # b stream entirely.
            finish_tensor(0, coss)
            fillers(4)
            nc.scalar.copy(out_sb["c0"][0:NCLS, :], out_cs[0][:, 0:H])
            nc.scalar.dma_start(out=out_c_cb[:, 0:H], in_=out_sb["c0"][:])
            nc.vector.tensor_copy(out_sb["c1"][0:NCLS, :], out_cs[0][:, H:B])
            nc.scalar.dma_start(out=out_c_cb[:, H:B], in_=out_sb["c1"][:])

            for gi, n in enumerate(GROUPS_B):
                load_group(1, gi, n)
            finish_tensor(1, sins)

            nc.scalar.copy(out_sb["s0"][0:NCLS, :], out_cs[1][:, 0:H])
            nc.scalar.dma_start(out=out_s_cb[:, 0:H], in_=out_sb["s0"][:])
            nc.vector.tensor_copy(out_sb["s1"][0:NCLS, :], out_cs[1][:, H:B])
            nc.sync.dma_start(out=out_s_cb[:, H:B], in_=out_sb["s1"][:])

    nc.compile()
    return nc


def _get_nc():
    if "nc" not in _STATE:
        _STATE["nc"] = _build()
    return _STATE["nc"]


def _prep_in_maps(x, a, b, fc_w):
    import ml_dtypes

    bf16 = ml_dtypes.bfloat16
    xf = np.asarray(x, dtype=np.float32).reshape(B, D)
    xtb = np.ascontiguousarray(xf.T).astype(bf16)  # [D, B] bf16
    a2 = np.asarray(a, dtype=np.float32).reshape(D, D).astype(bf16)
    b2 = np.asarray(b, dtype=np.float32).reshape(D, D).astype(bf16)
    fw = np.asarray(fc_w, dtype=np.float32)
    in_maps = []
    for m in range(NCORES):
        sl = slice(m * DW, (m + 1) * DW)
        im = {}
        for ti, (t2, sizes) in enumerate(
            ((a2, GROUPS_A), (b2, GROUPS_B))
        ):
            ts = np.zeros((DPAD, DW), dtype=bf16)
            ts[:D] = t2[:, sl]
            c0 = 0
            for gi, n in enumerate(sizes):
                blk = (
                    ts[c0 * ROWSP : (c0 + n) * ROWSP, :]
                    .reshape(n, ROWSP, DW)
                    .transpose(1, 0, 2)
                )
                im[f"{'ab'[ti]}{gi}"] = np.ascontiguousarray(blk).reshape(
                    ROWSP, n * DW
                )
                c0 += n
        xs = xtb[sl, :].reshape(NSUB, P, B).transpose(1, 0, 2)
        im["xt_s"] = np.ascontiguousarray(xs).reshape(P, NSUB * B)
        fs = np.ascontiguousarray(fw[:, sl].T).reshape(NSUB, P, NCLS)
        im["fwt_s"] = np.ascontiguousarray(
            fs.transpose(1, 0, 2).astype(bf16)
        ).reshape(P, NSUB * NCLS)
        in_maps.append(im)
    return in_maps


def _run(inputs, trace=False, trace_kwargs=None):
    """Run the device kernel; returns (final_output, BassKernelResults)."""
    from concourse.bass_utils import run_bass_kernel_spmd

    x = inputs["x"]
    a = inputs["a"]
    b = inputs["b"]
    w = np.asarray(inputs["w"], dtype=np.float64)
    n_param = np.asarray(inputs["n_param"], dtype=np.float64)
    fc_w = np.asarray(inputs["fc_w"], dtype=np.float32)
    fc_b = np.asarray(inputs["fc_b"], dtype=np.float32)

    nc = _get_nc()
    in_maps = _prep_in_maps(x, a, b, fc_w)
    res = run_bass_kernel_spmd(
        nc,
        in_maps,
        list(range(NCORES)),
        trace=trace,
        **(trace_kwargs or {}),
    )

    acc = np.zeros((NCLS, B), dtype=np.float32)
    for r in res.results:
        acc += np.asarray(r["out_c"][:NCLS], dtype=np.float32)
        acc += np.asarray(r["out_s"][:NCLS], dtype=np.float32)
    part1 = float(np.sum(w[1:] * n_param[1:] + w[:-1] * n_param[:-1]))
    final = acc.T + np.float32(part1) * fc_w.sum(axis=1)[None, :] + fc_b[None, :]
    return np.ascontiguousarray(final.astype(np.float32)), res


def kernel(**inputs) -> np.ndarray:
    out, _ = _run(inputs, trace=False)
    return out
